# revision 18
# baseline (speedup 1.0000x reference)
"""MultiHeadEMA (MEGA bidirectional EMA + residual + SiLU) on 8 Trainium2 cores.

Overlap-save convolution with DFT length F=256, tap support +-T=32
(kernel-tail L1 <= 1.1e-2, validated ~8e-4 end-to-end vs the fp64
reference), hop C=192, NW=22 windows.  E is sharded 128 channels/core;
the EMA parameters are folded on the host into frequency-domain
coefficient planes, and the omega*x residual is tap 0 of the kernel.

Spectrum packing (F=256 -> 256 real rows = 2 partition blocks):
    rows   0..127 : Re X[f], f=0..127             (PSUM pair [:, 0, :])
    rows 128..255 : Im X[f], f=1..127; row 128 = Re X[128]  ([:, 1, :])
Pointwise complex multiply = 5 engine ops per window; the sign of KIm is
folded into the planes and Q's halves are written swapped, so both
combines collapse into ONE fp16 2x-mode add:
    ev = fp16(pair)                ACT copy [128,(2,512)]
    P  = (KRe_dc | KRe_nyq) * ev   DVE 2x   [128,(2,512)]
    Q[:,0,:] = (-KIm) * ev_im      DVE 2x   [128,512]
    Q[:,1,:] = (+KIm) * ev_re      GPS      [128,512]
    Y  = P + Q                     DVE 2x   [128,(2,512)]  = (YRe | YIm)
DC (f=0) and Nyquist (row 128) come out right purely from plane row 0
(KIm row 0 is 0 for both halves).

Windows are paired for the inverse: a pair's 2*C=384 outputs fill 3 full
PSUM banks; the straddling middle bank is accumulated by both windows
using zero-padded half-V matrices, so SiLU runs as 3 full [128,512] ACT
ops and the pair leaves as ONE [128,3,512] fp16 DMA.

Window start 192c falls off the 128-row tile grid for odd c; odd windows
contract 3 x-tiles against zero-bordered W chunks instead of reloading a
shifted copy of x.
"""

import math
import numpy as np
from contextlib import ExitStack

import concourse.bass as bass
import concourse.tile as tile
from concourse import bacc, mybir
from concourse.bass_utils import run_bass_kernel_spmd

L, B, E, NDIM = 4096, 4, 1024, 16
N_CORES = 8
ESH = E // N_CORES            # 128 channels per core
F, T, C = 256, 32, 192        # DFT length, tap support, hop
NW = (L + C - 1) // C         # 22 windows
NP = (NW + 1) // 2            # 11 window pairs
FREE = B * ESH                # 512 free elements (b, chan)
NXT = 34                      # x tiles: padded rows [0, 4352), x at [T, T+L)

F16 = mybir.dt.float16
F32 = mybir.dt.float32

LAST_RESULTS = None
_CACHE: dict = {}


def _build_nc():
    nc = bacc.Bacc("TRN2", target_bir_lowering=False, debug=False,
                   num_devices=N_CORES)
    xs = nc.dram_tensor("xs", [NXT * 128, B, ESH], F16, kind="ExternalInput").ap()
    wfe = nc.dram_tensor("wfe", [2, 2, 128, 128], F16, kind="ExternalInput").ap()
    wfo = nc.dram_tensor("wfo", [3, 2, 128, 128], F16, kind="ExternalInput").ap()
    vi = nc.dram_tensor("vi", [4, 2, 128, 128], F16, kind="ExternalInput").ap()
    pl = nc.dram_tensor("pl", [8, 128, FREE], F16, kind="ExternalInput").ap()
    out = nc.dram_tensor("out", [L, B, ESH], F16, kind="ExternalOutput").ap()

    with ExitStack() as ctx:
        tc = ctx.enter_context(tile.TileContext(nc))
        cpool = ctx.enter_context(tc.tile_pool(name="const", bufs=1))
        ppool = ctx.enter_context(tc.tile_pool(name="pw", bufs=2))
        opool = ctx.enter_context(tc.tile_pool(name="outp", bufs=2))
        ps_f = ctx.enter_context(tc.tile_pool(name="psf", bufs=1, space="PSUM"))
        ps_r = ctx.enter_context(tc.tile_pool(name="psr", bufs=1, space="PSUM"))

        # DMA staging: window 0/1 need wfe + x tiles 0..3 first.  The x
        # bulk goes on the Sync HWDGE queue — the Scalar queue shares the
        # ACT engine, which is a near-critical compute resource.
        x_all = cpool.tile([128, NXT, FREE], F16)
        xr = xs.rearrange("(t p) b c -> p t (b c)", p=128)
        wfe_t = cpool.tile([128, 2, 2, 128], F16)
        nc.scalar.dma_start(wfe_t[:], wfe.transpose([2, 0, 1, 3]))
        nc.sync.dma_start(x_all[:, 0:2, :], xr[:, 0:2, :])
        nc.sync.dma_start(x_all[:, 2:4, :], xr[:, 2:4, :])
        wfo_t = cpool.tile([128, 3, 2, 128], F16)
        nc.scalar.dma_start(wfo_t[:], wfo.transpose([2, 0, 1, 3]))
        vi_t = cpool.tile([128, 4, 2, 128], F16)
        nc.scalar.dma_start(vi_t[:], vi.transpose([2, 0, 1, 3]))
        pl_t = cpool.tile([128, 8, FREE], F16)
        nc.scalar.dma_start(pl_t[:], pl.transpose([1, 0, 2]))
        for t0 in range(4, NXT, 6):
            t1 = min(t0 + 6, NXT)
            nc.sync.dma_start(x_all[:, t0:t1, :], xr[:, t0:t1, :])

        # PE p-state warm-up on a zeroed SBUF tile: no DMA dependency, so
        # the clock ramps while the first x tiles stream in.
        zb = cpool.tile([128, FREE], F16)
        nc.gpsimd.memset(zb[:], 0.0)
        warm = ps_f.tile([128, 2, FREE], F32, tag="fw", name="warm", bufs=2)
        for r in range(14):
            nc.tensor.matmul(warm[:, 0, :], zb[:, 0:128], zb[:],
                             start=(r == 0), stop=(r == 13))

        def fwd(c):
            """forward DFT of window c -> PSUM pair (XRe | XIm)"""
            pair = ps_f.tile([128, 2, FREE], F32, tag="fw", name=f"fw{c}",
                             bufs=2)
            if c % 2 == 0:
                g = 3 * c // 2
                chunks = [(wfe_t, 0, g), (wfe_t, 1, g + 1)]
            else:
                g = (3 * c - 1) // 2
                chunks = [(wfo_t, 0, g), (wfo_t, 1, g + 1), (wfo_t, 2, g + 2)]
            chunks = [ch for ch in chunks if ch[2] < 33]  # tile 33 is zero pad
            for blk in range(2):
                for i, (wt, k, t) in enumerate(chunks):
                    nc.tensor.matmul(
                        pair[:, blk, :],
                        wt[:, k, blk, :],
                        x_all[:, t, :],
                        start=(i == 0), stop=(i == len(chunks) - 1))
            return pair

        def evac(w, pair, ev):
            """PSUM pair -> fp16 halves of the pair-level ev tile.  ACT
            takes 3 of the 4 half-evacuations per pair, DVE one — that
            balances ACT (evs+silu) against DVE (evs+U+V_im+Y)."""
            nc.scalar.copy(ev[:, w, 0, :], pair[:, 0, :])
            if w == 0:
                nc.vector.tensor_copy(ev[:, w, 1, :], pair[:, 1, :])
            else:
                nc.scalar.copy(ev[:, w, 1, :], pair[:, 1, :])

        def pointwise_pair(p, ev):
            """complex multiply for both windows -> Y [128,(2,2,512)] fp16"""
            P = ppool.tile([128, 2, 2, FREE], F16, tag="P", name=f"P{p}")
            nc.vector.tensor_mul(P[:], ev[:], pl_t[:, 0:4, :]
                                 .rearrange("p (w k) f -> p w k f", w=2))
            Q = ppool.tile([128, 2, 2, FREE], F16, tag="Q", name=f"Q{p}")
            nc.gpsimd.tensor_mul(Q[:, 0, 0, :], ev[:, 0, 1, :], pl_t[:, 4, :])
            nc.vector.tensor_mul(Q[:, 1, 0, :], ev[:, 1, 1, :], pl_t[:, 5, :])
            nc.gpsimd.tensor_mul(Q[:, :, 1, :], ev[:, :, 0, :],
                                 pl_t[:, 6:8, :])
            Y = ppool.tile([128, 2, 2, FREE], F16, tag="Y", name=f"Y{p}")
            nc.vector.tensor_add(Y[:], P[:], Q[:])
            return Y

        def inv_pair(p, Y):
            """inverse DFT of pair p -> silu -> one out DMA"""
            last = p == NP - 1
            nblk = 2 if last else 3
            o = opool.tile([128, nblk, FREE], F16,
                           tag="olast" if last else "o", name=f"o{p}",
                           bufs=1 if last else 2)
            r0 = ps_r.tile([128, FREE], F32, tag="r0", name=f"r0_{p}", bufs=1)
            nc.tensor.matmul(r0[:], vi_t[:, 0, 0, :], Y[:, 0, 0, :],
                             start=True, stop=False)
            nc.tensor.matmul(r0[:], vi_t[:, 0, 1, :], Y[:, 0, 1, :],
                             start=False, stop=True)
            nc.scalar.activation(o[:, 0, :], r0[:],
                                 mybir.ActivationFunctionType.Silu)
            # straddling bank: window 2p rows 128..191 then 2p+1 rows 0..63
            r12 = ps_r.tile([128, 2, FREE], F32, tag="r12", name=f"r12_{p}",
                            bufs=1)
            nc.tensor.matmul(r12[:, 0, :], vi_t[:, 1, 0, :], Y[:, 0, 0, :],
                             start=True, stop=False)
            nc.tensor.matmul(r12[:, 0, :], vi_t[:, 1, 1, :], Y[:, 0, 1, :],
                             start=False, stop=False)
            nc.tensor.matmul(r12[:, 0, :], vi_t[:, 2, 0, :], Y[:, 1, 0, :],
                             start=False, stop=False)
            nc.tensor.matmul(r12[:, 0, :], vi_t[:, 2, 1, :], Y[:, 1, 1, :],
                             start=False, stop=True)
            if not last:
                nc.tensor.matmul(r12[:, 1, :], vi_t[:, 3, 0, :], Y[:, 1, 0, :],
                                 start=True, stop=False)
                nc.tensor.matmul(r12[:, 1, :], vi_t[:, 3, 1, :], Y[:, 1, 1, :],
                                 start=False, stop=True)
                nc.scalar.activation(o[:, 1:3, :], r12[:],
                                     mybir.ActivationFunctionType.Silu)
            else:
                nc.scalar.activation(o[:, 1, :], r12[:, 0, :],
                                     mybir.ActivationFunctionType.Silu)
            nc.sync.dma_start(
                out[2 * C * p: 2 * C * p + nblk * 128, :, :]
                .rearrange("(k p) b c -> p k (b c)", p=128),
                o[:])

        # software-skewed pipeline over window pairs: the next pair's
        # forward DFTs enter the PE queue right after this pair's PSUM
        # evacuations (which free the fwd banks), ahead of the pointwise
        # chain and this pair's inverse.  During pipeline fill the PE
        # would idle >3.4us waiting on the first elementwise chains —
        # that re-throttles the HAM clock gate to 1.2 GHz and the whole
        # core never recovers; dep-free heater matmuls keep it at 2.4.
        heat = ps_r.tile([128, FREE], F32, tag="heat", name="heat", bufs=1)
        pair0, pair1 = fwd(0), fwd(1)
        for p in range(NP):
            ev = ppool.tile([128, 2, 2, FREE], F16, tag="ev", name=f"ev{p}")
            evac(0, pair0, ev)
            evac(1, pair1, ev)
            pair0 = fwd(2 * p + 2) if 2 * p + 2 < NW else None
            pair1 = fwd(2 * p + 3) if 2 * p + 3 < NW else None
            nheat = {0: 16, 1: 24, 2: 10, 3: 6, 4: 4}.get(p, 0)
            for r in range(nheat):
                nc.tensor.matmul(heat[:], zb[:, 0:128], zb[:],
                                 start=(r == 0), stop=(r == nheat - 1))
            Y = pointwise_pair(p, ev)
            inv_pair(p, Y)
    nc.compile()
    return nc


def _host_prep(x, alpha, delta, beta, gamma, omega):
    """Fold EMA params into freq-domain planes + DFT matrices; shard x."""
    a = 1.0 / (1.0 + np.exp(-alpha.astype(np.float64)))
    d = 1.0 / (1.0 + np.exp(-delta.astype(np.float64)))
    q = 1.0 - a * d
    w = (a * beta.astype(np.float64))[:, :, 0] * gamma.astype(np.float64)
    w *= math.sqrt(1.0 / NDIM)
    tau = np.arange(T + 1)
    kern = (w[:, :, None] * q[:, :, 0:1] ** tau[None, None, :]).sum(1)  # (2E,T+1)
    k1, k2 = kern[:E], kern[E:]
    kc = np.zeros((E, F))
    kc[:, 0:T + 1] = k1
    kc[:, F - T:] += k2[:, :T][:, ::-1]      # slot F-i holds k2[i-1]
    kc[:, 0] += omega.astype(np.float64)     # residual: omega on tap 0
    Khat = np.fft.rfft(kc, axis=1)           # (E, 129)
    KRe, KIm = Khat.real, Khat.imag          # KIm[:,0] = KIm[:,128] = 0

    # coefficient planes [8, 128, E] (row = freq f, col = channel),
    # doubled for the pair-batched [2-window] pointwise ops:
    #   0-3: P planes  (KRe | KRe-with-Nyq-row0) x 2 windows
    #   4-5: Qhi = -KIm x 2   (DVE: ev_im -> Q[:, :, 0, :])
    #   6-7: Qlo = +KIm x 2   (GPS: ev_re -> Q[:, :, 1, :])
    planes = np.zeros((8, 128, E))
    fr = np.arange(128)
    pre = KRe[:, fr].T
    pim = KRe[:, fr].T.copy()
    pim[0] = KRe[:, 128]
    planes[0], planes[1], planes[2], planes[3] = pre, pim, pre, pim
    planes[4] = planes[5] = -KIm[:, fr].T
    planes[6] = planes[7] = KIm[:, fr].T

    # forward DFT, rows: 0..127 Re f, 128..255 Im f (row 128 = Nyquist Re)
    j = np.arange(128)
    m = np.arange(F)
    Wm = np.empty((F, F))
    Wm[0:128] = np.cos(2 * np.pi * np.outer(j, m) / F)
    Wm[128:256] = -np.sin(2 * np.pi * np.outer(j, m) / F)
    Wm[128] = np.cos(np.pi * m)
    # even-window chunks: wfe[k, blk, mlocal, r] = Wm[128 blk + r, 128 k + m]
    wfe = np.empty((2, 2, 128, 128))
    for k in range(2):
        for blk in range(2):
            wfe[k, blk] = Wm[128 * blk:128 * blk + 128, 128 * k:128 * k + 128].T
    # odd-window chunks (window offset 64 into first tile, zero borders)
    wfo = np.zeros((3, 2, 128, 128))
    for blk in range(2):
        Wb = Wm[128 * blk:128 * blk + 128]
        wfo[0, blk, 64:128] = Wb[:, 0:64].T       # tile row m -> pos m-64
        wfo[1, blk] = Wb[:, 64:192].T             # m -> 64+m
        wfo[2, blk, 0:64] = Wb[:, 192:256].T      # m -> 192+m
    # inverse: V[row, jout] over out positions j+T, packed-row convention
    jj = np.arange(C) + T
    V = np.empty((F, C))
    V[0:128] = 2 * np.cos(2 * np.pi * np.outer(j, jj) / F) / F
    V[0] = 1.0 / F
    V[128:256] = -2 * np.sin(2 * np.pi * np.outer(j, jj) / F) / F
    V[128] = ((-1.0) ** jj) / F
    # lhsT kinds: L1 = cols 0:128; L2 = [cols 128:192 | 0];
    #             L3 = [0 | cols 0:64]; L4 = cols 64:192
    vi = np.zeros((4, 2, 128, 128))
    for k in range(2):
        Vk = V[128 * k:128 * k + 128]
        vi[0, k] = Vk[:, 0:128]
        vi[1, k, :, 0:64] = Vk[:, 128:192]
        vi[2, k, :, 64:128] = Vk[:, 0:64]
        vi[3, k] = Vk[:, 64:192]

    xpad = np.zeros((NXT * 128, B, E), np.float16)
    xpad[T:T + L] = x.astype(np.float16)

    wfe16 = np.ascontiguousarray(wfe.astype(np.float16))
    wfo16 = np.ascontiguousarray(wfo.astype(np.float16))
    vi16 = np.ascontiguousarray(vi.astype(np.float16))
    in_maps = []
    for core in range(N_CORES):
        sl = slice(core * ESH, (core + 1) * ESH)
        plc = np.broadcast_to(
            planes.reshape(8, 128, 1, E)[:, :, :, sl],
            (8, 128, B, ESH)).reshape(8, 128, FREE)
        in_maps.append({
            "xs": np.ascontiguousarray(xpad[:, :, sl]),
            "wfe": wfe16,
            "wfo": wfo16,
            "vi": vi16,
            "pl": np.ascontiguousarray(plc.astype(np.float16)),
        })
    return in_maps


def kernel(x, alpha, delta, beta, gamma, omega):
    global LAST_RESULTS
    if "nc" not in _CACHE:
        _CACHE["nc"] = _build_nc()
    nc = _CACHE["nc"]
    in_maps = _host_prep(x, alpha, delta, beta, gamma, omega)
    res = run_bass_kernel_spmd(nc, in_maps, core_ids=list(range(N_CORES)))
    LAST_RESULTS = res
    out = np.concatenate([res.results[c]["out"] for c in range(N_CORES)], axis=2)
    return out.astype(np.float32)


# revision 21
# speedup vs baseline: 1.2121x; 1.2121x over previous
"""MultiHeadEMA (MEGA bidirectional EMA + residual + SiLU) on 8 Trainium2 cores.

Overlap-save convolution with DFT length F=256, tap support +-T=32
(kernel-tail L1 <= 1.1e-2, validated ~8e-4 end-to-end vs the fp64
reference), hop C=192, NW=22 windows.  E is sharded 128 channels/core;
the EMA parameters are folded on the host into frequency-domain
coefficient planes, and the omega*x residual is tap 0 of the kernel.

Spectrum packing (F=256 -> 256 real rows = 2 partition blocks):
    rows   0..127 : Re X[f], f=0..127             (PSUM pair [:, 0, :])
    rows 128..255 : Im X[f], f=1..127; row 128 = Re X[128]  ([:, 1, :])
Pointwise complex multiply = 5 engine ops per window; the sign of KIm is
folded into the planes and Q's halves are written swapped, so both
combines collapse into ONE fp16 2x-mode add:
    ev = fp16(pair)                ACT copy [128,(2,512)]
    P  = (KRe_dc | KRe_nyq) * ev   DVE 2x   [128,(2,512)]
    Q[:,0,:] = (-KIm) * ev_im      DVE 2x   [128,512]
    Q[:,1,:] = (+KIm) * ev_re      GPS      [128,512]
    Y  = P + Q                     DVE 2x   [128,(2,512)]  = (YRe | YIm)
DC (f=0) and Nyquist (row 128) come out right purely from plane row 0
(KIm row 0 is 0 for both halves).

Windows are paired for the inverse: a pair's 2*C=384 outputs fill 3 full
PSUM banks; the straddling middle bank is accumulated by both windows
using zero-padded half-V matrices, so SiLU runs as 3 full [128,512] ACT
ops and the pair leaves as ONE [128,3,512] fp16 DMA.

Window start 192c falls off the 128-row tile grid for odd c; odd windows
contract 3 x-tiles against zero-bordered W chunks instead of reloading a
shifted copy of x.
"""

import math
import numpy as np
from contextlib import ExitStack

import concourse.bass as bass
import concourse.tile as tile
from concourse import bacc, mybir
from concourse.bass_utils import run_bass_kernel_spmd

L, B, E, NDIM = 4096, 4, 1024, 16
N_CORES = 8
ESH = E // N_CORES            # 128 channels per core
F, T, C = 256, 32, 192        # DFT length, tap support, hop
NW = (L + C - 1) // C         # 22 windows
NP = (NW + 1) // 2            # 11 window pairs
FREE = B * ESH                # 512 free elements (b, chan)
NXT = 34                      # x tiles: padded rows [0, 4352), x at [T, T+L)

F16 = mybir.dt.float16
F32 = mybir.dt.float32

LAST_RESULTS = None
_CACHE: dict = {}


def _build_nc():
    nc = bacc.Bacc("TRN2", target_bir_lowering=False, debug=False,
                   num_devices=N_CORES)
    xs = nc.dram_tensor("xs", [NXT * 128, B, ESH], F16, kind="ExternalInput").ap()
    # consts are stored in DRAM already in SBUF layout (partition-major)
    # so their load DMAs are fully contiguous per partition.
    wfe = nc.dram_tensor("wfe", [128, 2, 2, 128], F16, kind="ExternalInput").ap()
    wfo = nc.dram_tensor("wfo", [128, 3, 2, 128], F16, kind="ExternalInput").ap()
    vi = nc.dram_tensor("vi", [128, 4, 2, 128], F16, kind="ExternalInput").ap()
    pl = nc.dram_tensor("pl", [128, 8, FREE], F16, kind="ExternalInput").ap()
    out = nc.dram_tensor("out", [L, B, ESH], F16, kind="ExternalOutput").ap()

    with ExitStack() as ctx:
        tc = ctx.enter_context(tile.TileContext(nc))
        cpool = ctx.enter_context(tc.tile_pool(name="const", bufs=1))
        ppool = ctx.enter_context(tc.tile_pool(name="pw", bufs=2))
        opool = ctx.enter_context(tc.tile_pool(name="outp", bufs=2))
        ps_f = ctx.enter_context(tc.tile_pool(name="psf", bufs=1, space="PSUM"))
        ps_r = ctx.enter_context(tc.tile_pool(name="psr", bufs=1, space="PSUM"))

        # DMA staging: window 0/1 need wfe + x tiles 0..3 first.  The x
        # bulk goes on the Sync HWDGE queue — the Scalar queue shares the
        # ACT engine, which is a near-critical compute resource.
        x_all = cpool.tile([128, NXT, FREE], F16)
        xr = xs.rearrange("(t p) b c -> p t (b c)", p=128)
        wfe_t = cpool.tile([128, 2, 2, 128], F16)
        nc.scalar.dma_start(wfe_t[:], wfe)
        nc.sync.dma_start(x_all[:, 0:2, :], xr[:, 0:2, :])
        nc.sync.dma_start(x_all[:, 2:4, :], xr[:, 2:4, :])
        wfo_t = cpool.tile([128, 3, 2, 128], F16)
        nc.scalar.dma_start(wfo_t[:], wfo)
        pl_t = cpool.tile([128, 8, FREE], F16)
        nc.scalar.dma_start(pl_t[:], pl)
        vi_t = cpool.tile([128, 4, 2, 128], F16)
        nc.scalar.dma_start(vi_t[:], vi)
        for t0 in range(4, NXT, 6):
            t1 = min(t0 + 6, NXT)
            nc.sync.dma_start(x_all[:, t0:t1, :], xr[:, t0:t1, :])

        # PE p-state warm-up on a zeroed SBUF tile: no DMA dependency, so
        # the clock ramps while the first x tiles stream in.
        zb = cpool.tile([128, FREE], F16)
        nc.gpsimd.memset(zb[:], 0.0)
        warm = ps_f.tile([128, 2, FREE], F32, tag="fw", name="warm", bufs=2)
        for r in range(14):
            nc.tensor.matmul(warm[:, 0, :], zb[:, 0:128], zb[:],
                             start=(r == 0), stop=(r == 13))

        def fwd(c):
            """forward DFT of window c -> PSUM pair (XRe | XIm)"""
            pair = ps_f.tile([128, 2, FREE], F32, tag="fw", name=f"fw{c}",
                             bufs=2)
            if c % 2 == 0:
                g = 3 * c // 2
                chunks = [(wfe_t, 0, g), (wfe_t, 1, g + 1)]
            else:
                g = (3 * c - 1) // 2
                chunks = [(wfo_t, 0, g), (wfo_t, 1, g + 1), (wfo_t, 2, g + 2)]
            chunks = [ch for ch in chunks if ch[2] < 33]  # tile 33 is zero pad
            for blk in range(2):
                for i, (wt, k, t) in enumerate(chunks):
                    nc.tensor.matmul(
                        pair[:, blk, :],
                        wt[:, k, blk, :],
                        x_all[:, t, :],
                        start=(i == 0), stop=(i == len(chunks) - 1))
            return pair

        def evac(w, pair, ev):
            """PSUM pair -> fp16 pair-level ev tile, one ACT op per window
            (ACT: 4 evs + 1 silu vs DVE: U+V_im+Y balances out)."""
            nc.scalar.copy(ev[:, w, :, :], pair[:])

        def pointwise_pair(p, ev):
            """complex multiply for both windows -> Y [128,(2,2,512)] fp16"""
            P = ppool.tile([128, 2, 2, FREE], F16, tag="P", name=f"P{p}")
            nc.vector.tensor_mul(P[:], ev[:], pl_t[:, 0:4, :]
                                 .rearrange("p (w k) f -> p w k f", w=2))
            Q = ppool.tile([128, 2, 2, FREE], F16, tag="Q", name=f"Q{p}")
            nc.vector.tensor_mul(Q[:, :, 0, :], ev[:, :, 1, :],
                                 pl_t[:, 4:6, :])
            nc.gpsimd.tensor_mul(Q[:, :, 1, :], ev[:, :, 0, :],
                                 pl_t[:, 6:8, :])
            Y = ppool.tile([128, 2, 2, FREE], F16, tag="Y", name=f"Y{p}")
            nc.vector.tensor_add(Y[:], P[:], Q[:])
            return Y

        def inv_pair(p, Y):
            """inverse DFT of pair p -> silu -> one out DMA"""
            last = p == NP - 1
            nblk = 2 if last else 3
            o = opool.tile([128, nblk, FREE], F16,
                           tag="olast" if last else "o", name=f"o{p}",
                           bufs=1 if last else 2)
            r0 = ps_r.tile([128, FREE], F32, tag="r0", name=f"r0_{p}", bufs=1)
            nc.tensor.matmul(r0[:], vi_t[:, 0, 0, :], Y[:, 0, 0, :],
                             start=True, stop=False)
            nc.tensor.matmul(r0[:], vi_t[:, 0, 1, :], Y[:, 0, 1, :],
                             start=False, stop=True)
            nc.scalar.activation(o[:, 0, :], r0[:],
                                 mybir.ActivationFunctionType.Silu)
            # straddling bank: window 2p rows 128..191 then 2p+1 rows 0..63
            r12 = ps_r.tile([128, 2, FREE], F32, tag="r12", name=f"r12_{p}",
                            bufs=1)
            nc.tensor.matmul(r12[:, 0, :], vi_t[:, 1, 0, :], Y[:, 0, 0, :],
                             start=True, stop=False)
            nc.tensor.matmul(r12[:, 0, :], vi_t[:, 1, 1, :], Y[:, 0, 1, :],
                             start=False, stop=False)
            nc.tensor.matmul(r12[:, 0, :], vi_t[:, 2, 0, :], Y[:, 1, 0, :],
                             start=False, stop=False)
            nc.tensor.matmul(r12[:, 0, :], vi_t[:, 2, 1, :], Y[:, 1, 1, :],
                             start=False, stop=True)
            if not last:
                nc.tensor.matmul(r12[:, 1, :], vi_t[:, 3, 0, :], Y[:, 1, 0, :],
                                 start=True, stop=False)
                nc.tensor.matmul(r12[:, 1, :], vi_t[:, 3, 1, :], Y[:, 1, 1, :],
                                 start=False, stop=True)
                nc.scalar.activation(o[:, 1:3, :], r12[:],
                                     mybir.ActivationFunctionType.Silu)
            else:
                nc.scalar.activation(o[:, 1, :], r12[:, 0, :],
                                     mybir.ActivationFunctionType.Silu)
            nc.sync.dma_start(
                out[2 * C * p: 2 * C * p + nblk * 128, :, :]
                .rearrange("(k p) b c -> p k (b c)", p=128),
                o[:])

        # software-skewed pipeline over window pairs: the next pair's
        # forward DFTs enter the PE queue right after this pair's PSUM
        # evacuations (which free the fwd banks), ahead of the pointwise
        # chain and this pair's inverse.  During pipeline fill the PE
        # would idle >3.4us waiting on the first elementwise chains —
        # that re-throttles the HAM clock gate to 1.2 GHz and the whole
        # core never recovers; dep-free heater matmuls keep it at 2.4.
        heat = ps_r.tile([128, FREE], F32, tag="heat", name="heat", bufs=1)
        pair0, pair1 = fwd(0), fwd(1)
        for p in range(NP):
            ev = ppool.tile([128, 2, 2, FREE], F16, tag="ev", name=f"ev{p}")
            evac(0, pair0, ev)
            evac(1, pair1, ev)
            pair0 = fwd(2 * p + 2) if 2 * p + 2 < NW else None
            pair1 = fwd(2 * p + 3) if 2 * p + 3 < NW else None
            nheat = {0: 16, 1: 16, 2: 8, 3: 4}.get(p, 0)
            for r in range(nheat):
                nc.tensor.matmul(heat[:], zb[:, 0:128], zb[:],
                                 start=(r == 0), stop=(r == nheat - 1))
            Y = pointwise_pair(p, ev)
            inv_pair(p, Y)
    nc.compile()
    return nc


def _host_prep(x, alpha, delta, beta, gamma, omega):
    """Fold EMA params into freq-domain planes + DFT matrices; shard x."""
    a = 1.0 / (1.0 + np.exp(-alpha.astype(np.float64)))
    d = 1.0 / (1.0 + np.exp(-delta.astype(np.float64)))
    q = 1.0 - a * d
    w = (a * beta.astype(np.float64))[:, :, 0] * gamma.astype(np.float64)
    w *= math.sqrt(1.0 / NDIM)
    tau = np.arange(T + 1)
    kern = (w[:, :, None] * q[:, :, 0:1] ** tau[None, None, :]).sum(1)  # (2E,T+1)
    k1, k2 = kern[:E], kern[E:]
    kc = np.zeros((E, F))
    kc[:, 0:T + 1] = k1
    kc[:, F - T:] += k2[:, :T][:, ::-1]      # slot F-i holds k2[i-1]
    kc[:, 0] += omega.astype(np.float64)     # residual: omega on tap 0
    Khat = np.fft.rfft(kc, axis=1)           # (E, 129)
    KRe, KIm = Khat.real, Khat.imag          # KIm[:,0] = KIm[:,128] = 0

    # coefficient planes [8, 128, E] (row = freq f, col = channel),
    # doubled for the pair-batched [2-window] pointwise ops:
    #   0-3: P planes  (KRe | KRe-with-Nyq-row0) x 2 windows
    #   4-5: Qhi = -KIm x 2   (DVE: ev_im -> Q[:, :, 0, :])
    #   6-7: Qlo = +KIm x 2   (GPS: ev_re -> Q[:, :, 1, :])
    planes = np.zeros((8, 128, E))
    fr = np.arange(128)
    pre = KRe[:, fr].T
    pim = KRe[:, fr].T.copy()
    pim[0] = KRe[:, 128]
    planes[0], planes[1], planes[2], planes[3] = pre, pim, pre, pim
    planes[4] = planes[5] = -KIm[:, fr].T
    planes[6] = planes[7] = KIm[:, fr].T

    # forward DFT, rows: 0..127 Re f, 128..255 Im f (row 128 = Nyquist Re)
    j = np.arange(128)
    m = np.arange(F)
    Wm = np.empty((F, F))
    Wm[0:128] = np.cos(2 * np.pi * np.outer(j, m) / F)
    Wm[128:256] = -np.sin(2 * np.pi * np.outer(j, m) / F)
    Wm[128] = np.cos(np.pi * m)
    # even-window chunks: wfe[k, blk, mlocal, r] = Wm[128 blk + r, 128 k + m]
    wfe = np.empty((2, 2, 128, 128))
    for k in range(2):
        for blk in range(2):
            wfe[k, blk] = Wm[128 * blk:128 * blk + 128, 128 * k:128 * k + 128].T
    # odd-window chunks (window offset 64 into first tile, zero borders)
    wfo = np.zeros((3, 2, 128, 128))
    for blk in range(2):
        Wb = Wm[128 * blk:128 * blk + 128]
        wfo[0, blk, 64:128] = Wb[:, 0:64].T       # tile row m -> pos m-64
        wfo[1, blk] = Wb[:, 64:192].T             # m -> 64+m
        wfo[2, blk, 0:64] = Wb[:, 192:256].T      # m -> 192+m
    # inverse: V[row, jout] over out positions j+T, packed-row convention
    jj = np.arange(C) + T
    V = np.empty((F, C))
    V[0:128] = 2 * np.cos(2 * np.pi * np.outer(j, jj) / F) / F
    V[0] = 1.0 / F
    V[128:256] = -2 * np.sin(2 * np.pi * np.outer(j, jj) / F) / F
    V[128] = ((-1.0) ** jj) / F
    # lhsT kinds: L1 = cols 0:128; L2 = [cols 128:192 | 0];
    #             L3 = [0 | cols 0:64]; L4 = cols 64:192
    vi = np.zeros((4, 2, 128, 128))
    for k in range(2):
        Vk = V[128 * k:128 * k + 128]
        vi[0, k] = Vk[:, 0:128]
        vi[1, k, :, 0:64] = Vk[:, 128:192]
        vi[2, k, :, 64:128] = Vk[:, 0:64]
        vi[3, k] = Vk[:, 64:192]

    xpad = np.zeros((NXT * 128, B, E), np.float16)
    xpad[T:T + L] = x.astype(np.float16)

    # DRAM layouts match SBUF (partition-major) so load DMAs are linear
    wfe16 = np.ascontiguousarray(wfe.transpose(2, 0, 1, 3).astype(np.float16))
    wfo16 = np.ascontiguousarray(wfo.transpose(2, 0, 1, 3).astype(np.float16))
    vi16 = np.ascontiguousarray(vi.transpose(2, 0, 1, 3).astype(np.float16))
    in_maps = []
    for core in range(N_CORES):
        sl = slice(core * ESH, (core + 1) * ESH)
        plc = np.broadcast_to(
            planes.reshape(8, 128, 1, E)[:, :, :, sl],
            (8, 128, B, ESH)).reshape(8, 128, FREE)
        in_maps.append({
            "xs": np.ascontiguousarray(xpad[:, :, sl]),
            "wfe": wfe16,
            "wfo": wfo16,
            "vi": vi16,
            "pl": np.ascontiguousarray(
                plc.transpose(1, 0, 2).astype(np.float16)),
        })
    return in_maps


def kernel(x, alpha, delta, beta, gamma, omega):
    global LAST_RESULTS
    if "nc" not in _CACHE:
        _CACHE["nc"] = _build_nc()
    nc = _CACHE["nc"]
    in_maps = _host_prep(x, alpha, delta, beta, gamma, omega)
    res = run_bass_kernel_spmd(nc, in_maps, core_ids=list(range(N_CORES)))
    LAST_RESULTS = res
    out = np.concatenate([res.results[c]["out"] for c in range(N_CORES)], axis=2)
    return out.astype(np.float32)


# revision 23
# speedup vs baseline: 1.2198x; 1.0063x over previous
"""MultiHeadEMA (MEGA bidirectional EMA + residual + SiLU) on 8 Trainium2 cores.

Overlap-save convolution with DFT length F=256, tap support +-T=32
(kernel-tail L1 <= 1.1e-2, validated ~8e-4 end-to-end vs the fp64
reference), hop C=192, NW=22 windows.  E is sharded 128 channels/core;
the EMA parameters are folded on the host into frequency-domain
coefficient planes, and the omega*x residual is tap 0 of the kernel.

Spectrum packing (F=256 -> 256 real rows = 2 partition blocks):
    rows   0..127 : Re X[f], f=0..127             (PSUM pair [:, 0, :])
    rows 128..255 : Im X[f], f=1..127; row 128 = Re X[128]  ([:, 1, :])
Pointwise complex multiply = 5 engine ops per window; the sign of KIm is
folded into the planes and Q's halves are written swapped, so both
combines collapse into ONE fp16 2x-mode add:
    ev = fp16(pair)                ACT copy [128,(2,512)]
    P  = (KRe_dc | KRe_nyq) * ev   DVE 2x   [128,(2,512)]
    Q[:,0,:] = (-KIm) * ev_im      DVE 2x   [128,512]
    Q[:,1,:] = (+KIm) * ev_re      GPS      [128,512]
    Y  = P + Q                     DVE 2x   [128,(2,512)]  = (YRe | YIm)
DC (f=0) and Nyquist (row 128) come out right purely from plane row 0
(KIm row 0 is 0 for both halves).

Windows are paired for the inverse: a pair's 2*C=384 outputs fill 3 full
PSUM banks; the straddling middle bank is accumulated by both windows
using zero-padded half-V matrices, so SiLU runs as 3 full [128,512] ACT
ops and the pair leaves as ONE [128,3,512] fp16 DMA.

Window start 192c falls off the 128-row tile grid for odd c; odd windows
contract 3 x-tiles against zero-bordered W chunks instead of reloading a
shifted copy of x.
"""

import math
import numpy as np
from contextlib import ExitStack

import concourse.bass as bass
import concourse.tile as tile
from concourse import bacc, mybir
from concourse.bass_utils import run_bass_kernel_spmd

L, B, E, NDIM = 4096, 4, 1024, 16
N_CORES = 8
ESH = E // N_CORES            # 128 channels per core
F, T, C = 256, 32, 192        # DFT length, tap support, hop
NW = (L + C - 1) // C         # 22 windows
NP = (NW + 1) // 2            # 11 window pairs
FREE = B * ESH                # 512 free elements (b, chan)
NXT = 34                      # x tiles: padded rows [0, 4352), x at [T, T+L)

F16 = mybir.dt.float16
F32 = mybir.dt.float32

LAST_RESULTS = None
_CACHE: dict = {}


def _build_nc():
    nc = bacc.Bacc("TRN2", target_bir_lowering=False, debug=False,
                   num_devices=N_CORES)
    xs = nc.dram_tensor("xs", [NXT * 128, B, ESH], F16, kind="ExternalInput").ap()
    # consts are stored in DRAM already in SBUF layout (partition-major)
    # so their load DMAs are fully contiguous per partition.
    wfe = nc.dram_tensor("wfe", [128, 2, 2, 128], F16, kind="ExternalInput").ap()
    wfo = nc.dram_tensor("wfo", [128, 3, 2, 128], F16, kind="ExternalInput").ap()
    vi = nc.dram_tensor("vi", [128, 4, 2, 128], F16, kind="ExternalInput").ap()
    pl = nc.dram_tensor("pl", [128, 8, FREE], F16, kind="ExternalInput").ap()
    out = nc.dram_tensor("out", [L, B, ESH], F16, kind="ExternalOutput").ap()

    with ExitStack() as ctx:
        tc = ctx.enter_context(tile.TileContext(nc))
        cpool = ctx.enter_context(tc.tile_pool(name="const", bufs=1))
        ppool = ctx.enter_context(tc.tile_pool(name="pw", bufs=2))
        opool = ctx.enter_context(tc.tile_pool(name="outp", bufs=2))
        ps_f = ctx.enter_context(tc.tile_pool(name="psf", bufs=1, space="PSUM"))
        ps_r = ctx.enter_context(tc.tile_pool(name="psr", bufs=1, space="PSUM"))

        # DMA staging: window 0/1 need wfe + x tiles 0..3 first.  The x
        # bulk goes on the Sync HWDGE queue — the Scalar queue shares the
        # ACT engine, which is a near-critical compute resource.
        x_all = cpool.tile([128, NXT, FREE], F16)
        xr = xs.rearrange("(t p) b c -> p t (b c)", p=128)
        wfe_t = cpool.tile([128, 2, 2, 128], F16)
        nc.scalar.dma_start(wfe_t[:], wfe)
        nc.sync.dma_start(x_all[:, 0:2, :], xr[:, 0:2, :])
        nc.sync.dma_start(x_all[:, 2:4, :], xr[:, 2:4, :])
        wfo_t = cpool.tile([128, 3, 2, 128], F16)
        nc.scalar.dma_start(wfo_t[:], wfo)
        pl_t = cpool.tile([128, 8, FREE], F16)
        nc.scalar.dma_start(pl_t[:], pl)
        vi_t = cpool.tile([128, 4, 2, 128], F16)
        nc.scalar.dma_start(vi_t[:], vi)
        for t0 in range(4, NXT, 6):
            t1 = min(t0 + 6, NXT)
            nc.sync.dma_start(x_all[:, t0:t1, :], xr[:, t0:t1, :])

        # PE p-state warm-up on a zeroed SBUF tile: no DMA dependency, so
        # the clock ramps while the first x tiles stream in.
        zb = cpool.tile([128, FREE], F16)
        nc.gpsimd.memset(zb[:], 0.0)
        warm = ps_f.tile([128, 2, FREE], F32, tag="fw", name="warm", bufs=2)
        for r in range(14):
            nc.tensor.matmul(warm[:, 0, :], zb[:, 0:128], zb[:],
                             start=(r == 0), stop=(r == 13))

        def fwd(c):
            """forward DFT of window c -> PSUM pair (XRe | XIm)"""
            pair = ps_f.tile([128, 2, FREE], F32, tag="fw", name=f"fw{c}",
                             bufs=2)
            if c % 2 == 0:
                g = 3 * c // 2
                chunks = [(wfe_t, 0, g), (wfe_t, 1, g + 1)]
            else:
                g = (3 * c - 1) // 2
                chunks = [(wfo_t, 0, g), (wfo_t, 1, g + 1), (wfo_t, 2, g + 2)]
            chunks = [ch for ch in chunks if ch[2] < 33]  # tile 33 is zero pad
            for blk in range(2):
                for i, (wt, k, t) in enumerate(chunks):
                    nc.tensor.matmul(
                        pair[:, blk, :],
                        wt[:, k, blk, :],
                        x_all[:, t, :],
                        start=(i == 0), stop=(i == len(chunks) - 1))
            return pair

        def evac(w, pair, ev):
            """PSUM pair -> fp16 pair-level ev tile, one ACT op per window.
            ev is half-major [128, half, win, FREE] so the downstream
            P/Q ops read fully contiguous [1024]/[2048] spans."""
            nc.scalar.copy(ev[:, :, w, :], pair[:])

        def pointwise_pair(p, ev):
            """complex multiply for both windows -> Y [128,(2,2,512)] fp16"""
            P = ppool.tile([128, 2, 2, FREE], F16, tag="P", name=f"P{p}")
            nc.vector.tensor_mul(P[:], ev[:], pl_t[:, 0:4, :]
                                 .rearrange("p (h w) f -> p h w f", h=2))
            Q = ppool.tile([128, 2, 2, FREE], F16, tag="Q", name=f"Q{p}")
            nc.vector.tensor_mul(Q[:, 0, :, :], ev[:, 1, :, :],
                                 pl_t[:, 4:6, :])
            nc.gpsimd.tensor_mul(Q[:, 1, :, :], ev[:, 0, :, :],
                                 pl_t[:, 6:8, :])
            Y = ppool.tile([128, 2, 2, FREE], F16, tag="Y", name=f"Y{p}")
            nc.vector.tensor_add(Y[:], P[:], Q[:])
            return Y

        def inv_mms(p, Y):
            """inverse DFT matmuls of pair p -> (r0, r12) PSUM banks"""
            last = p == NP - 1
            r0 = ps_r.tile([128, FREE], F32, tag="r0", name=f"r0_{p}", bufs=1)
            nc.tensor.matmul(r0[:], vi_t[:, 0, 0, :], Y[:, 0, 0, :],
                             start=True, stop=False)
            nc.tensor.matmul(r0[:], vi_t[:, 0, 1, :], Y[:, 1, 0, :],
                             start=False, stop=True)
            # straddling bank: window 2p rows 128..191 then 2p+1 rows 0..63
            r12 = ps_r.tile([128, 2, FREE], F32, tag="r12", name=f"r12_{p}",
                            bufs=1)
            nc.tensor.matmul(r12[:, 0, :], vi_t[:, 1, 0, :], Y[:, 0, 0, :],
                             start=True, stop=False)
            nc.tensor.matmul(r12[:, 0, :], vi_t[:, 1, 1, :], Y[:, 1, 0, :],
                             start=False, stop=False)
            nc.tensor.matmul(r12[:, 0, :], vi_t[:, 2, 0, :], Y[:, 0, 1, :],
                             start=False, stop=False)
            nc.tensor.matmul(r12[:, 0, :], vi_t[:, 2, 1, :], Y[:, 1, 1, :],
                             start=False, stop=True)
            if not last:
                nc.tensor.matmul(r12[:, 1, :], vi_t[:, 3, 0, :], Y[:, 0, 1, :],
                                 start=True, stop=False)
                nc.tensor.matmul(r12[:, 1, :], vi_t[:, 3, 1, :], Y[:, 1, 1, :],
                                 start=False, stop=True)
            return r0, r12

        def finish(p, r0, r12):
            """silu + output DMA for pair p (emitted one pair late so the
            ACT queue serves the next pair's evacuations first)"""
            last = p == NP - 1
            nblk = 2 if last else 3
            o = opool.tile([128, nblk, FREE], F16,
                           tag="olast" if last else "o", name=f"o{p}",
                           bufs=1 if last else 2)
            nc.scalar.activation(o[:, 0, :], r0[:],
                                 mybir.ActivationFunctionType.Silu)
            if not last:
                nc.scalar.activation(o[:, 1:3, :], r12[:],
                                     mybir.ActivationFunctionType.Silu)
            else:
                nc.scalar.activation(o[:, 1, :], r12[:, 0, :],
                                     mybir.ActivationFunctionType.Silu)
            nc.sync.dma_start(
                out[2 * C * p: 2 * C * p + nblk * 128, :, :]
                .rearrange("(k p) b c -> p k (b c)", p=128),
                o[:])

        # software-skewed pipeline over window pairs.  Queue-order rules:
        # - next pair's fwd right after this pair's evacuations (fw WAR)
        # - heater matmuls bridge PE idle during pipeline fill (HAM)
        # - finish(p-1) AFTER evac(p) on the ACT queue, BEFORE inv(p) on
        #   the PE queue (keeps single-buffered r banks legal)
        heat = ps_r.tile([128, FREE], F32, tag="heat", name="heat", bufs=1)
        pair0, pair1 = fwd(0), fwd(1)
        pend = None
        for p in range(NP):
            ev = ppool.tile([128, 2, 2, FREE], F16, tag="ev", name=f"ev{p}")
            evac(0, pair0, ev)
            evac(1, pair1, ev)
            pair0 = fwd(2 * p + 2) if 2 * p + 2 < NW else None
            pair1 = fwd(2 * p + 3) if 2 * p + 3 < NW else None
            nheat = {0: 16, 1: 16, 2: 8, 3: 4}.get(p, 0)
            for r in range(nheat):
                nc.tensor.matmul(heat[:], zb[:, 0:128], zb[:],
                                 start=(r == 0), stop=(r == nheat - 1))
            Y = pointwise_pair(p, ev)
            if pend is not None:
                finish(*pend)
            pend = (p,) + inv_mms(p, Y)
        finish(*pend)
    nc.compile()
    return nc


def _host_prep(x, alpha, delta, beta, gamma, omega):
    """Fold EMA params into freq-domain planes + DFT matrices; shard x."""
    a = 1.0 / (1.0 + np.exp(-alpha.astype(np.float64)))
    d = 1.0 / (1.0 + np.exp(-delta.astype(np.float64)))
    q = 1.0 - a * d
    w = (a * beta.astype(np.float64))[:, :, 0] * gamma.astype(np.float64)
    w *= math.sqrt(1.0 / NDIM)
    tau = np.arange(T + 1)
    kern = (w[:, :, None] * q[:, :, 0:1] ** tau[None, None, :]).sum(1)  # (2E,T+1)
    k1, k2 = kern[:E], kern[E:]
    kc = np.zeros((E, F))
    kc[:, 0:T + 1] = k1
    kc[:, F - T:] += k2[:, :T][:, ::-1]      # slot F-i holds k2[i-1]
    kc[:, 0] += omega.astype(np.float64)     # residual: omega on tap 0
    Khat = np.fft.rfft(kc, axis=1)           # (E, 129)
    KRe, KIm = Khat.real, Khat.imag          # KIm[:,0] = KIm[:,128] = 0

    # coefficient planes [8, 128, E] (row = freq f, col = channel),
    # doubled for the pair-batched [2-window] pointwise ops:
    #   0-3: P planes  (KRe | KRe-with-Nyq-row0) x 2 windows
    #   4-5: Qhi = -KIm x 2   (DVE: ev_im -> Q[:, :, 0, :])
    #   6-7: Qlo = +KIm x 2   (GPS: ev_re -> Q[:, :, 1, :])
    planes = np.zeros((8, 128, E))
    fr = np.arange(128)
    pre = KRe[:, fr].T
    pim = KRe[:, fr].T.copy()
    pim[0] = KRe[:, 128]
    planes[0], planes[1], planes[2], planes[3] = pre, pre, pim, pim
    planes[4] = planes[5] = -KIm[:, fr].T
    planes[6] = planes[7] = KIm[:, fr].T

    # forward DFT, rows: 0..127 Re f, 128..255 Im f (row 128 = Nyquist Re)
    j = np.arange(128)
    m = np.arange(F)
    Wm = np.empty((F, F))
    Wm[0:128] = np.cos(2 * np.pi * np.outer(j, m) / F)
    Wm[128:256] = -np.sin(2 * np.pi * np.outer(j, m) / F)
    Wm[128] = np.cos(np.pi * m)
    # even-window chunks: wfe[k, blk, mlocal, r] = Wm[128 blk + r, 128 k + m]
    wfe = np.empty((2, 2, 128, 128))
    for k in range(2):
        for blk in range(2):
            wfe[k, blk] = Wm[128 * blk:128 * blk + 128, 128 * k:128 * k + 128].T
    # odd-window chunks (window offset 64 into first tile, zero borders)
    wfo = np.zeros((3, 2, 128, 128))
    for blk in range(2):
        Wb = Wm[128 * blk:128 * blk + 128]
        wfo[0, blk, 64:128] = Wb[:, 0:64].T       # tile row m -> pos m-64
        wfo[1, blk] = Wb[:, 64:192].T             # m -> 64+m
        wfo[2, blk, 0:64] = Wb[:, 192:256].T      # m -> 192+m
    # inverse: V[row, jout] over out positions j+T, packed-row convention
    jj = np.arange(C) + T
    V = np.empty((F, C))
    V[0:128] = 2 * np.cos(2 * np.pi * np.outer(j, jj) / F) / F
    V[0] = 1.0 / F
    V[128:256] = -2 * np.sin(2 * np.pi * np.outer(j, jj) / F) / F
    V[128] = ((-1.0) ** jj) / F
    # lhsT kinds: L1 = cols 0:128; L2 = [cols 128:192 | 0];
    #             L3 = [0 | cols 0:64]; L4 = cols 64:192
    vi = np.zeros((4, 2, 128, 128))
    for k in range(2):
        Vk = V[128 * k:128 * k + 128]
        vi[0, k] = Vk[:, 0:128]
        vi[1, k, :, 0:64] = Vk[:, 128:192]
        vi[2, k, :, 64:128] = Vk[:, 0:64]
        vi[3, k] = Vk[:, 64:192]

    xpad = np.zeros((NXT * 128, B, E), np.float16)
    xpad[T:T + L] = x.astype(np.float16)

    # DRAM layouts match SBUF (partition-major) so load DMAs are linear
    wfe16 = np.ascontiguousarray(wfe.transpose(2, 0, 1, 3).astype(np.float16))
    wfo16 = np.ascontiguousarray(wfo.transpose(2, 0, 1, 3).astype(np.float16))
    vi16 = np.ascontiguousarray(vi.transpose(2, 0, 1, 3).astype(np.float16))
    in_maps = []
    for core in range(N_CORES):
        sl = slice(core * ESH, (core + 1) * ESH)
        plc = np.broadcast_to(
            planes.reshape(8, 128, 1, E)[:, :, :, sl],
            (8, 128, B, ESH)).reshape(8, 128, FREE)
        in_maps.append({
            "xs": np.ascontiguousarray(xpad[:, :, sl]),
            "wfe": wfe16,
            "wfo": wfo16,
            "vi": vi16,
            "pl": np.ascontiguousarray(
                plc.transpose(1, 0, 2).astype(np.float16)),
        })
    return in_maps


def kernel(x, alpha, delta, beta, gamma, omega):
    global LAST_RESULTS
    if "nc" not in _CACHE:
        _CACHE["nc"] = _build_nc()
    nc = _CACHE["nc"]
    in_maps = _host_prep(x, alpha, delta, beta, gamma, omega)
    res = run_bass_kernel_spmd(nc, in_maps, core_ids=list(range(N_CORES)))
    LAST_RESULTS = res
    out = np.concatenate([res.results[c]["out"] for c in range(N_CORES)], axis=2)
    return out.astype(np.float32)
